# revision 9
# baseline (speedup 1.0000x reference)
"""Trainium2 Bass kernel for per-position head-attention (nn_DariushFlashAttention2).

Math (per batch b, sequence position s):
    Q = q[b,s].reshape(H=32, D=128); K, V likewise
    logits = Q @ K.T / sqrt(D)          # [32, 32] attention over HEADS
    W = softmax(logits, axis=-1)
    out[b,s] = (W @ V).reshape(H*D)

Every one of the B*S = 8192 positions is independent, so we shard positions
across the 8 NeuronCores (1024 positions each) and run one SPMD program.

v4 design (per core):
  - q,k cast to fp8 e3m4 on host (1 B/elem, fp16 matmul speed, end-to-end
    rel-err 1.66e-2 < 2e-2 gate); v and out stay fp16.  The kernel is
    DMA-bound (~72us of queue time/core), so bytes are the roofline.
  - Positions packed 4-per-group on the 128 partitions; host pre-transposes
    q,k into [d, (pos,h)] and concatenates them into one dram tensor.
  - Work unit is a "quad" = 4 groups = 16 positions:
      QK (PE, col-tiled per position) -> exp (ScalarE, [128,128]) ->
      WV (PE, diagonal-tiled; V carries a ones column so the softmax
      denominator lands in the psum) -> normalize-while-evacuating
      (ScalarE activation-with-scale / VectorE tensor_scalar, split to
      balance the two engines; DVE reciprocal for the denominators).
  - Quads are SOFTWARE-PIPELINED: the PE stream is QK(i+1), WV(i), ... so
    the PE never stalls on ScalarE latency and stays HAM-warm at 2.4 GHz.
  - Output halves drain via the Scalar HWDGE ring (inputs prefetch on the
    Sync ring) so out-DMAs never head-of-line-block input prefetch.
"""

import numpy as np

B, S, H, D = 2, 4096, 32, 128
NCORES = 8
POS = B * S                  # 8192 positions total
PPC = POS // NCORES          # 1024 positions per core
GP = 4                       # positions per group (4*32 heads = 128 partitions)
NG = 16                      # groups per chunk
CHUNK_POS = GP * NG          # 64 positions per chunk
NCHUNK = PPC // CHUNK_POS    # 16 chunks per core
NGD = NG * D                 # 2048 q (or k) columns per chunk
VCOL = D + 1                 # v columns per group incl. ones column
NQ = NCHUNK * 4              # quads per core

_SCALE = float(1.0 / np.sqrt(D))

_program = None  # cached compiled Bass program


def _build_program():
    import concourse.bacc as bacc
    import concourse.mybir as mybir
    from concourse.tile import TileContext

    fp32 = mybir.dt.float32
    fp16 = mybir.dt.float16
    fp8 = mybir.dt.float8e3

    nc = bacc.Bacc()
    qk = nc.dram_tensor("qk", [NCHUNK, 128, 2 * NGD], fp8, kind="ExternalInput")
    vp = nc.dram_tensor("vp", [NCHUNK, 128, NG * VCOL], fp16, kind="ExternalInput")
    out = nc.dram_tensor("out", [NCHUNK, 128, NGD], fp16, kind="ExternalOutput")

    with TileContext(nc) as tc:
        with (
            tc.tile_pool(name="qk_in", bufs=4) as qk_pool,
            tc.tile_pool(name="v_in", bufs=4) as v_pool,
            tc.tile_pool(name="o_out", bufs=3) as o_pool,
            tc.tile_pool(name="exp", bufs=3) as exp_pool,
            tc.tile_pool(name="small", bufs=8) as small_pool,
            tc.tile_pool(name="psl", bufs=3, space="PSUM") as psl_pool,
            tc.tile_pool(name="pso", bufs=4, space="PSUM") as pso_pool,
        ):
            chunk_tiles = {}   # n -> (qk_t, vp_t, out_t)
            st = {}            # quad index -> exp tile

            def stage_qk(i):
                n, q = divmod(i, 4)
                if q == 0:
                    qk_t = qk_pool.tile([128, 2 * NGD], fp8, tag="qk")
                    nc.sync.dma_start(out=qk_t, in_=qk[n])
                    vp_t = v_pool.tile([128, NG * VCOL], fp16, tag="vp")
                    nc.sync.dma_start(out=vp_t, in_=vp[n])
                    out_t = o_pool.tile([128, NGD], fp16, tag="out")
                    chunk_tiles[n] = (qk_t, vp_t, out_t)
                qk_t, _, _ = chunk_tiles[n]
                psl = psl_pool.tile([128, 128], fp32, tag="psl")
                for t in range(4):
                    g = q * 4 + t
                    for j in range(GP):
                        c0 = g * D + 32 * j
                        nc.tensor.matmul(
                            psl[32 * j:32 * j + 32, 32 * t:32 * t + 32],
                            qk_t[:, NGD + c0:NGD + c0 + 32],   # K stationary
                            qk_t[:, c0:c0 + 32],               # Q moving
                            start=True, stop=True,
                            tile_position=(0, 32 * j),
                        )
                exp_sb = exp_pool.tile([128, 128], fp16, tag="exp")
                nc.scalar.activation(
                    exp_sb, psl, mybir.ActivationFunctionType.Exp, scale=_SCALE)
                st[i] = exp_sb

            def stage_wv(i):
                n, q = divmod(i, 4)
                exp_sb = st.pop(i)
                _, vp_t, out_t = chunk_tiles[n]
                for p2 in range(2):              # pair of groups
                    pso = pso_pool.tile([128, 2 * VCOL], fp32, tag="pso")
                    for u in range(2):
                        t = p2 * 2 + u
                        g = q * 4 + t
                        for j in range(GP):
                            r = slice(32 * j, 32 * j + 32)
                            nc.tensor.matmul(
                                pso[r, u * VCOL:(u + 1) * VCOL],
                                exp_sb[r, 32 * t:32 * t + 32],
                                vp_t[r, g * VCOL:(g + 1) * VCOL],
                                start=True, stop=True,
                                tile_position=(32 * j, 32 * j),
                            )
                    recip = small_pool.tile([128, 2], fp32, tag="recip")
                    zcols = pso.rearrange("p (u c) -> p u c", c=VCOL)[:, :, D]
                    nc.vector.reciprocal(recip, zcols)
                    for u in range(2):
                        g = q * 4 + p2 * 2 + u
                        src = pso[:, u * VCOL:u * VCOL + D]
                        dst = out_t[:, g * D:(g + 1) * D]
                        if g % 16 < 7:
                            nc.scalar.activation(
                                dst, src, mybir.ActivationFunctionType.Copy,
                                scale=recip[:, u:u + 1],
                            )
                        else:
                            nc.vector.tensor_scalar_mul(dst, src, recip[:, u:u + 1])

                # Drain finished halves early on the Scalar HWDGE ring.
                if q == 1:
                    nc.scalar.dma_start(
                        out=out[n, :, :NGD // 2], in_=out_t[:, :NGD // 2])
                elif q == 3:
                    nc.scalar.dma_start(
                        out=out[n, :, NGD // 2:], in_=out_t[:, NGD // 2:])

            for i in range(NQ + 1):
                if i < NQ:
                    stage_qk(i)
                if i >= 1:
                    stage_wv(i - 1)

    nc.compile()
    return nc


def _host_pack(q, k, v):
    """Build per-core device input arrays from full fp32 inputs."""
    import ml_dtypes
    f8 = ml_dtypes.float8_e3m4

    qf = np.ascontiguousarray(q, dtype=np.float32).reshape(POS, H, D)
    kf = np.ascontiguousarray(k, dtype=np.float32).reshape(POS, H, D)
    vf = np.ascontiguousarray(v, dtype=np.float32).reshape(POS, H, D)

    nchunk_tot = POS // CHUNK_POS
    # q,k: [chunk, group, i, h, d] -> [chunk, d, (group, i, h)]
    def to_qt(x):
        x = x.reshape(nchunk_tot, NG, GP, H, D)
        x = x.transpose(0, 4, 1, 2, 3)
        return x.reshape(nchunk_tot, D, NG * GP * H)

    qk_all = np.concatenate([to_qt(qf), to_qt(kf)], axis=2)
    qk_all = np.ascontiguousarray(qk_all).astype(f8)

    # v: [chunk, group, i, gh, d] -> [chunk, (i,gh), (group, d|1)]
    vv = vf.reshape(nchunk_tot, NG, GP, H, D).transpose(0, 2, 3, 1, 4)
    vp_all = np.ones((nchunk_tot, GP, H, NG, VCOL), dtype=np.float32)
    vp_all[..., :D] = vv
    vp_all = np.ascontiguousarray(
        vp_all.reshape(nchunk_tot, GP * H, NG * VCOL)
    ).astype(np.float16)

    in_maps = []
    for c in range(NCORES):
        sl = slice(c * NCHUNK, (c + 1) * NCHUNK)
        in_maps.append({
            "qk": np.ascontiguousarray(qk_all[sl]),
            "vp": np.ascontiguousarray(vp_all[sl]),
        })
    return in_maps


def _host_unpack(outs):
    """Per-core [NCHUNK, 128, NG*D] fp16 -> full [B, S, H*D] fp32."""
    full = np.concatenate(outs, axis=0).astype(np.float32)
    nchunk_tot = POS // CHUNK_POS
    full = full.reshape(nchunk_tot, GP, H, NG, D)   # [chunk, i, h, g, d]
    full = full.transpose(0, 3, 1, 2, 4)            # [chunk, g, i, h, d]
    return np.ascontiguousarray(full.reshape(B, S, H * D))


def kernel(q, k, v, _trace=False):
    global _program
    from concourse.bass_utils import run_bass_kernel_spmd

    if _program is None:
        _program = _build_program()

    in_maps = _host_pack(q, k, v)
    res = run_bass_kernel_spmd(_program, in_maps, list(range(NCORES)), trace=_trace)
    outs = [res.results[c]["out"] for c in range(NCORES)]
    result = _host_unpack(outs)
    if _trace:
        return result, res
    return result
